# revision 1
# baseline (speedup 1.0000x reference)
"""Trainium2 Bass kernel for a bidirectional selective-scan SSM (Mamba-like).

Problem: nn_ProMU_42623255445559
  B=8, L=2048, D=256, N=16, R=16
  Data-parallel over batch: core i handles batch row i; weights replicated.

Per-core dataflow (compute tensors transposed: d on partitions, l in free):
  x_dbl^T  = Wxp~ @ x^T            (PE; Bf/Bb rows sign-flipped host-side)
  -delta^T = ln(sigmoid(-(W_dt @ delta_r^T + b_dt)))   (PE + ACT, 2 passes)
  a_n      = exp((-delta^T) * exp(A_log)[:,n])         (ACT, fused scale)
  b_n      = (-delta*x)^T*(-Bf_n) + (-delta_b*xf)^T*(-Bb_n)    (DVE)
  h_n      = hw scan along l: h = a*h + b              (DVE tensor_tensor_scan)
  y^T      = sum_n h_n * C_n  +  D_skip*(x^T + xf^T)
  out      = y @ W_out^T           (PE, lhsT = y^T chunks, rhs = W_out^T)

Host-side prep in kernel(): weight transposes, -b_dt, exp(A_log), sign flips.
"""

import sys

sys.path.insert(0, "/opt/trn_rl_repo")

from contextlib import ExitStack

import numpy as np

import concourse.bacc as bacc
import concourse.bass as bass
import concourse.mybir as mybir
import concourse.tile as tile
from concourse import bass_utils, library_config
from concourse.bass import AP

B, L, D, N, R = 8, 2048, 256, 16, 16
PROJ = R + 3 * N  # 64 rows of x_dbl^T
FP32 = mybir.dt.float32
BF16 = mybir.dt.bfloat16
AF = mybir.ActivationFunctionType
ALU = mybir.AluOpType

NCORES = 8
LC = 256          # l-chunk for the scan pipeline
NLC = L // LC     # 8
NG = 4            # n per group
G = N // NG       # 4 groups
LSUB = 128        # l-subchunk for out-proj matmuls


def _rev_ap(ap2d):
    """Reverse the (single) free dim of a [P, F] AP."""
    (pstep, pcount), (fstep, fcount) = ap2d.ap
    assert fstep == 1
    return AP(ap2d.tensor, ap2d.offset + fcount - 1, [[pstep, pcount], [-1, fcount]])


def _rep_ap(ap2d, r):
    """Repeat a [P, F] AP r times along free -> [P, r, F] with stride 0."""
    (pstep, pcount), (fstep, fcount) = ap2d.ap
    assert fstep == 1
    return AP(ap2d.tensor, ap2d.offset, [[pstep, pcount], [0, r], [1, fcount]])


def _blk_ap(ap2d, r, f):
    """View a [P, r*f] AP as [P, r, f]."""
    (pstep, pcount), (fstep, fcount) = ap2d.ap
    assert fstep == 1 and fcount == r * f
    return AP(ap2d.tensor, ap2d.offset, [[pstep, pcount], [f, r], [1, f]])


def _cols_ap(ap2d, start, step, count):
    """Strided column gather: [P, count] picking cols start, start+step, ..."""
    (pstep, pcount), (fstep, fcount) = ap2d.ap
    assert fstep == 1
    return AP(ap2d.tensor, ap2d.offset + start, [[pstep, pcount], [step, count]])


def _emit(tc, nc, io):
    x_d, wxpT_d, wxbT_d, wdtT_d, mbdt_d, aexp_d, dskip_d, woutT_d, eye_d, out_d = io

    ctx = ExitStack()
    with ctx:
        const = ctx.enter_context(tc.tile_pool(name="const", bufs=1))
        big = ctx.enter_context(tc.tile_pool(name="big", bufs=1))
        tps = ctx.enter_context(tc.tile_pool(name="tps", bufs=2, space="PSUM"))
        mmp = ctx.enter_context(tc.tile_pool(name="mmp", bufs=2, space="PSUM"))
        ops = ctx.enter_context(tc.tile_pool(name="ops", bufs=2, space="PSUM"))
        ldp = ctx.enter_context(tc.tile_pool(name="ldp", bufs=3))
        wk = ctx.enter_context(tc.tile_pool(name="wk", bufs=2))

        # ---- constants (all pre-transposed host-side) ------------------
        eye = const.tile([128, 128], FP32, tag="eye")
        nc.sync.dma_start(eye[:, :], eye_d[:, :])

        wxpT = [const.tile([128, PROJ], FP32, name=f"wxpT{h}", tag=f"wxpT{h}")
                for h in range(2)]
        wxbT = [const.tile([128, R], FP32, name=f"wxbT{h}", tag=f"wxbT{h}")
                for h in range(2)]
        woutT = [const.tile([128, D], FP32, name=f"woutT{h}", tag=f"woutT{h}")
                 for h in range(2)]
        aexp = [const.tile([128, N], FP32, name=f"aexp{h}", tag=f"aexp{h}")
                for h in range(2)]
        mbdt = [const.tile([128, 1], FP32, name=f"mbdt{h}", tag=f"mbdt{h}")
                for h in range(2)]
        dskip = [const.tile([128, 1], FP32, name=f"dsk{h}", tag=f"dsk{h}")
                 for h in range(2)]
        for h in range(2):
            hs = slice(h * 128, (h + 1) * 128)
            nc.sync.dma_start(wxpT[h][:, :], wxpT_d[hs, :])
            nc.sync.dma_start(wxbT[h][:, :], wxbT_d[hs, :])
            nc.sync.dma_start(woutT[h][:, :], woutT_d[hs, :])
            nc.sync.dma_start(aexp[h][:, :], aexp_d[hs, :])
            nc.sync.dma_start(mbdt[h][:, :], mbdt_d[hs, :])
            nc.sync.dma_start(dskip[h][:, :], dskip_d[hs, :])
        wdtT = const.tile([R, D], FP32, tag="wdtT")
        nc.sync.dma_start(wdtT[:, :], wdtT_d[:, :])

        # pre-touch DMA'd weights on PE so later matmuls don't accumulate
        # more sync-wait commands than the ISA allows
        warm = tps.tile([128, 128], FP32, tag="tps")
        nc.tensor.transpose(warm[:, :], eye[:, :], eye[:, :])
        warm2 = tps.tile([PROJ, 128], FP32, tag="tps")
        nc.tensor.matmul(warm2[:, :], wxpT[0][:, :], eye[:, :],
                         start=True, stop=True)

        # carry state between l-chunks, per half: (128, N)
        carry = [const.tile([128, N], FP32, name=f"carry{h}", tag=f"carry{h}")
                 for h in range(2)]

        # ---- x^T and xf^T --------------------------------------------
        xT = [big.tile([128, L], FP32, name=f"xT{h}", tag=f"xT{h}") for h in range(2)]
        for i in range(L // 128):
            xn = ldp.tile([128, D], FP32, tag="ld256")
            nc.sync.dma_start(xn[:, :], x_d[i * 128:(i + 1) * 128, :])
            for h in range(2):
                pt = tps.tile([128, 128], FP32, tag="tps")
                nc.tensor.transpose(pt[:, :], xn[:, h * 128:(h + 1) * 128], eye[:, :])
                nc.scalar.copy(xT[h][:, i * 128:(i + 1) * 128], pt[:, :])

        xfT = [big.tile([128, L], FP32, name=f"xfT{h}", tag=f"xfT{h}")
               for h in range(2)]
        for h in range(2):
            nc.vector.tensor_copy(xfT[h][:, :], _rev_ap(xT[h][:, :]))

        # ---- projections ---------------------------------------------
        # x_dbl^T (64, L) = Wxp~ @ x^T   (Bf/Bb rows already negated)
        xdblT = big.tile([PROJ, L], FP32, tag="xdblT")
        for c in range(NLC):
            pt = mmp.tile([PROJ, LC], FP32, tag="mmp")
            for h in range(2):
                nc.tensor.matmul(pt[:, :], wxpT[h][:, :], xT[h][:, c * LC:(c + 1) * LC],
                                 start=(h == 0), stop=(h == 1))
            nc.scalar.copy(xdblT[:, c * LC:(c + 1) * LC], pt[:, :])

        # xb^T (16, L) = W_xbproj @ xf^T
        xbT = big.tile([R, L], FP32, tag="xbT")
        for c in range(NLC):
            pt = mmp.tile([R, LC], FP32, tag="mmp")
            for h in range(2):
                nc.tensor.matmul(pt[:, :], wxbT[h][:, :], xfT[h][:, c * LC:(c + 1) * LC],
                                 start=(h == 0), stop=(h == 1))
            nc.scalar.copy(xbT[:, c * LC:(c + 1) * LC], pt[:, :])

        # bf16 copy of Bf/Bb/C rows for 2x-mode elementwise work
        xdbl16 = big.tile([PROJ, L], BF16, tag="xdbl16")
        nc.vector.tensor_copy(xdbl16[:, :], xdblT[:, :])

        # mdelta^T = -delta^T = ln(sigmoid(-(W_dt @ delta_r^T + b_dt)))
        # u = mdelta^T * x^T ; ub = mdelta_b^T * xf^T   (signs cancel with -Bf/-Bb)
        mdT = [big.tile([128, L], FP32, name=f"mdT{h}", tag=f"mdT{h}")
               for h in range(2)]
        ubT = [big.tile([128, L], BF16, name=f"ubT{h}", tag=f"ubT{h}")
               for h in range(2)]
        uT = [big.tile([128, L], BF16, name=f"uT{h}", tag=f"uT{h}") for h in range(2)]
        for h in range(2):
            for c in range(NLC):
                sl = slice(c * LC, (c + 1) * LC)
                pt = mmp.tile([128, LC], FP32, tag="mmp")
                nc.tensor.matmul(pt[:, :], wdtT[:, h * 128:(h + 1) * 128],
                                 xdblT[0:R, sl], start=True, stop=True)
                sg = wk.tile([128, LC], FP32, tag="sgc")
                nc.scalar.activation(sg[:, :], pt[:, :], AF.Sigmoid,
                                     bias=mbdt[h][:, 0:1], scale=-1.0)
                nc.scalar.activation(mdT[h][:, sl], sg[:, :], AF.Ln)
                pt2 = mmp.tile([128, LC], FP32, tag="mmp")
                nc.tensor.matmul(pt2[:, :], wdtT[:, h * 128:(h + 1) * 128],
                                 xbT[:, sl], start=True, stop=True)
                sg2 = wk.tile([128, LC], FP32, tag="sgc")
                nc.scalar.activation(sg2[:, :], pt2[:, :], AF.Sigmoid,
                                     bias=mbdt[h][:, 0:1], scale=-1.0)
                db = wk.tile([128, LC], FP32, tag="dbc")
                nc.scalar.activation(db[:, :], sg2[:, :], AF.Ln)
                nc.vector.tensor_mul(ubT[h][:, sl], db[:, :], xfT[h][:, sl])
            nc.vector.tensor_mul(uT[h][:, :], mdT[h][:, :], xT[h][:, :])

        # ---- main scan loop ------------------------------------------
        for c in range(NLC):
            sl = slice(c * LC, (c + 1) * LC)
            y_acc = [None, None]
            for g in range(G):
                n0 = g * NG
                bf_rep = wk.tile([128, NG * LC], BF16, tag="bfr")
                bb_rep = wk.tile([128, NG * LC], BF16, tag="bbr")
                c_rep = wk.tile([128, NG * LC], BF16, tag="ccr")
                # engine reads need 32-aligned partition starts; DMA rows
                # into flat partition-0 staging tiles first
                bf_fl = wk.tile([1, NG * LC], BF16, tag="bff", bufs=1)
                bb_fl = wk.tile([1, NG * LC], BF16, tag="bbf", bufs=1)
                c_fl = wk.tile([1, NG * LC], BF16, tag="ccf", bufs=1)
                nc.sync.dma_start(_blk_ap(bf_fl[0:1, :], NG, LC),
                                  xdbl16[R + n0:R + n0 + NG, sl])
                nc.sync.dma_start(_blk_ap(bb_fl[0:1, :], NG, LC),
                                  xdbl16[R + N + n0:R + N + n0 + NG, sl])
                nc.sync.dma_start(_blk_ap(c_fl[0:1, :], NG, LC),
                                  xdbl16[R + 2 * N + n0:R + 2 * N + n0 + NG, sl])
                for rep, fl in ((bf_rep, bf_fl), (bb_rep, bb_fl), (c_rep, c_fl)):
                    s = fl[0:1, :]
                    src_b = AP(s.tensor, s.offset,
                               [[s.ap[0][0], 1], [0, 128], [1, NG * LC]])
                    nc.sync.dma_start(rep[:, :], src_b)
                for h in range(2):
                    a_t = wk.tile([128, NG * LC], FP32, tag="at")
                    for j in range(NG):
                        n = n0 + j
                        nc.scalar.activation(a_t[:, j * LC:(j + 1) * LC],
                                             mdT[h][:, sl], AF.Exp,
                                             scale=aexp[h][:, n:n + 1])
                    p_t = wk.tile([128, NG * LC], BF16, tag="pt")
                    b_t = wk.tile([128, NG * LC], BF16, tag="bt")
                    nc.vector.tensor_tensor(_blk_ap(p_t[:, :], NG, LC),
                                            _rep_ap(uT[h][:, sl], NG),
                                            _blk_ap(bf_rep[:, :], NG, LC), ALU.mult)
                    nc.vector.tensor_tensor(_blk_ap(b_t[:, :], NG, LC),
                                            _rep_ap(ubT[h][:, sl], NG),
                                            _blk_ap(bb_rep[:, :], NG, LC), ALU.mult)
                    nc.vector.tensor_add(b_t[:, :], b_t[:, :], p_t[:, :])
                    h_t = wk.tile([128, NG * LC], BF16, tag="ht", bufs=3)
                    for j in range(NG):
                        n = n0 + j
                        js = slice(j * LC, (j + 1) * LC)
                        init = 0.0 if c == 0 else carry[h][:, n:n + 1]
                        nc.vector.tensor_tensor_scan(h_t[:, js], a_t[:, js],
                                                     b_t[:, js], init,
                                                     ALU.mult, ALU.add)
                    nc.scalar.copy(carry[h][:, n0:n0 + NG],
                                   _cols_ap(h_t[:, :], LC - 1, LC, NG))
                    tmp = wk.tile([128, NG * LC], BF16, tag="pt")
                    nc.vector.tensor_mul(tmp[:, :], h_t[:, :], c_rep[:, :])
                    fa = wk.tile([128, LC], BF16, tag="fa")
                    fb = wk.tile([128, LC], BF16, tag="fb")
                    nc.vector.tensor_add(fa[:, :], tmp[:, 0:LC], tmp[:, LC:2 * LC])
                    nc.vector.tensor_add(fb[:, :], tmp[:, 2 * LC:3 * LC],
                                         tmp[:, 3 * LC:4 * LC])
                    if y_acc[h] is None:
                        y_acc[h] = wk.tile([128, LC], FP32, name="yac", tag="ya",
                                           bufs=4)
                        nc.vector.tensor_add(y_acc[h][:, :], fa[:, :], fb[:, :])
                    else:
                        nc.vector.tensor_add(fa[:, :], fa[:, :], fb[:, :])
                        nc.vector.tensor_add(y_acc[h][:, :], y_acc[h][:, :],
                                             fa[:, :])
            # skip connection + out projection for this l-chunk
            y_fin = []
            for h in range(2):
                xs = wk.tile([128, LC], FP32, tag="xs")
                nc.vector.tensor_add(xs[:, :], xT[h][:, sl], xfT[h][:, sl])
                yf = wk.tile([128, LC], FP32, name=f"yf{h}", tag=f"yf{h}")
                nc.vector.scalar_tensor_tensor(yf[:, :], xs[:, :], dskip[h][:, 0:1],
                                               y_acc[h][:, :], ALU.mult, ALU.add)
                y_fin.append(yf)
            for s in range(LC // LSUB):
                l0 = c * LC + s * LSUB
                pt = ops.tile([LSUB, D], FP32, tag="ops")
                for h in range(2):
                    nc.tensor.matmul(pt[:, :],
                                     y_fin[h][:, s * LSUB:(s + 1) * LSUB],
                                     woutT[h][:, :], start=(h == 0), stop=(h == 1))
                ot = wk.tile([LSUB, D], FP32, tag="osb")
                nc.scalar.copy(ot[:, :], pt[:, :])
                nc.sync.dma_start(out_d[l0:l0 + LSUB, :], ot[:, :])


_NC_CACHE = {}  # v2 bf16


def _build():
    if "nc" in _NC_CACHE:
        return _NC_CACHE["nc"]
    nc = bacc.Bacc("TRN2", target_bir_lowering=False, debug=False,
                   num_devices=NCORES)
    x_d = nc.dram_tensor("x", [L, D], FP32, kind="ExternalInput").ap()
    wxpT_d = nc.dram_tensor("WxpT", [D, PROJ], FP32, kind="ExternalInput").ap()
    wxbT_d = nc.dram_tensor("WxbT", [D, R], FP32, kind="ExternalInput").ap()
    wdtT_d = nc.dram_tensor("WdtT", [R, D], FP32, kind="ExternalInput").ap()
    mbdt_d = nc.dram_tensor("mbdt", [D, 1], FP32, kind="ExternalInput").ap()
    aexp_d = nc.dram_tensor("Aexp", [D, N], FP32, kind="ExternalInput").ap()
    dskip_d = nc.dram_tensor("Dskip", [D, 1], FP32, kind="ExternalInput").ap()
    woutT_d = nc.dram_tensor("WoutT", [D, D], FP32, kind="ExternalInput").ap()
    eye_d = nc.dram_tensor("eye", [128, 128], FP32, kind="ExternalInput").ap()
    out_d = nc.dram_tensor("out", [L, D], FP32, kind="ExternalOutput").ap()
    io = (x_d, wxpT_d, wxbT_d, wdtT_d, mbdt_d, aexp_d, dskip_d, woutT_d,
          eye_d, out_d)
    with tile.TileContext(nc) as tc:
        _emit(tc, nc, io)
    nc.compile()
    _NC_CACHE["nc"] = nc
    return nc


def host_prep(W_xproj, W_xbproj, W_dt, b_dt, A_log, D_skip, W_out):
    """Host-side input transforms shared by all cores."""
    wxp = np.asarray(W_xproj, dtype=np.float32).copy()
    wxp[R:R + 2 * N, :] *= -1.0          # fold sign of -delta into Bf/Bb
    return {
        "WxpT": np.ascontiguousarray(wxp.T),
        "WxbT": np.ascontiguousarray(np.asarray(W_xbproj, dtype=np.float32).T),
        "WdtT": np.ascontiguousarray(np.asarray(W_dt, dtype=np.float32).T),
        "mbdt": np.ascontiguousarray(
            -np.asarray(b_dt, dtype=np.float32).reshape(D, 1)),
        "Aexp": np.ascontiguousarray(
            np.exp(np.asarray(A_log, dtype=np.float32))),
        "Dskip": np.ascontiguousarray(
            np.asarray(D_skip, dtype=np.float32).reshape(D, 1)),
        "WoutT": np.ascontiguousarray(np.asarray(W_out, dtype=np.float32).T),
        "eye": np.eye(128, dtype=np.float32),
    }


def kernel(x, W_xproj, W_xbproj, W_dt, b_dt, A_log, D_skip, W_out, **profile_kw):
    nc = _build()
    shared = host_prep(W_xproj, W_xbproj, W_dt, b_dt, A_log, D_skip, W_out)
    xs = np.asarray(x, dtype=np.float32)
    in_maps = [{"x": np.ascontiguousarray(xs[b]), **shared} for b in range(NCORES)]
    res = bass_utils.run_bass_kernel_spmd(nc, in_maps, core_ids=list(range(NCORES)),
                                          **profile_kw)
    out = np.stack([res.results[b]["out"] for b in range(NCORES)], axis=0)
    kernel.last_result = res
    return out



# revision 7
# speedup vs baseline: 1.4510x; 1.4510x over previous
"""Trainium2 Bass kernel for a bidirectional selective-scan SSM (Mamba-like).

Problem: nn_ProMU_42623255445559
  B=8, L=2048, D=256, N=16, R=16
  Data-parallel over batch: core i handles batch row i; weights replicated.

v3 dataflow (d on partitions, l in free; two 128-partition halves):
  x_dbl^T = Wxp @ x^T                  (PE)
  delta   = softplus(Wdt @ delta_r^T + b_dt) = ln(exp(z)+1)   (ACT exp+ln,
            single act-func table: ln/exp/copy/identity share set 6)
  delta_b computed in FORWARD order from x (not xf); consumers read it with
            reversed APs, so xf^T is never materialized.
  a_n     = exp(A_n * delta)           (ACT, per-partition scale = A_n < 0)
  b_n     = u*Bf_n + ub_rev*Bb_n       (DVE bf16 2x; u=delta*x, ub=delta_b*x)
  h_n     = scan(a, b) along l         (Pool engine; DVE stays on mults)
  yg      = tree-reduce_n (h_n * C_n)  (DVE bf16 2x, per n-group of 8)
  out     = (yg0 + yg1 + (x+xf)*D_skip) @ W_out^T
            -- assembled in PSUM: 6 accumulating bf16 matmuls (PE)

Host-side prep: weight transposes, A=-exp(A_log), +b_dt, bf16 W_out.
"""

import sys

sys.path.insert(0, "/opt/trn_rl_repo")

from contextlib import ExitStack

import numpy as np

import concourse.bacc as bacc
import concourse.bass as bass
import concourse.mybir as mybir
import concourse.tile as tile
from concourse import bass_utils
from concourse.bass import AP

B, L, D, N, R = 8, 2048, 256, 16, 16
PROJ = R + 3 * N  # 64 rows of x_dbl^T
FP32 = mybir.dt.float32
BF16 = mybir.dt.bfloat16
AF = mybir.ActivationFunctionType
ALU = mybir.AluOpType

NCORES = 8
LC = 512          # l-chunk for the scan pipeline
NLC = L // LC     # 4
NG = 8            # n per group
G = N // NG       # 2 groups
LSUB = 128        # l-subchunk for out-proj matmuls

# which (c, g, h) iterations run their reduce tree on Pool (balance tuning)
TREE_POOL = {(c, g, h) for c in range(NLC) for g in range(G) for h in range(2)}
# scans are DVE-only (TPB ISA rejects the scan opcode on Pool)
SCAN_POOL = set()


def _rev_ap(ap2d):
    """Reverse the (single) free dim of a [P, F] AP."""
    (pstep, pcount), (fstep, fcount) = ap2d.ap
    assert fstep == 1
    return AP(ap2d.tensor, ap2d.offset + fcount - 1, [[pstep, pcount], [-1, fcount]])


def _rep_ap(ap2d, r):
    """Repeat a [P, F] AP r times along free -> [P, r, F] with stride 0."""
    (pstep, pcount), (fstep, fcount) = ap2d.ap
    assert fstep == 1
    return AP(ap2d.tensor, ap2d.offset, [[pstep, pcount], [0, r], [1, fcount]])


def _rep_rev_ap(ap2d, r):
    """Repeat the REVERSED [P, F] AP r times along free -> [P, r, F]."""
    (pstep, pcount), (fstep, fcount) = ap2d.ap
    assert fstep == 1
    return AP(ap2d.tensor, ap2d.offset + fcount - 1,
              [[pstep, pcount], [0, r], [-1, fcount]])


def _blk_ap(ap2d, r, f):
    """View a [P, r*f] AP as [P, r, f]."""
    (pstep, pcount), (fstep, fcount) = ap2d.ap
    assert fstep == 1 and fcount == r * f
    return AP(ap2d.tensor, ap2d.offset, [[pstep, pcount], [f, r], [1, f]])


def _emit(tc, nc, io):
    x_d, wxpT_d, wxbT_d, wdtT_d, bdt_d, aneg_d, dskip_d, woutT_d, eye_d, out_d = io

    ctx = ExitStack()
    with ctx:
        const = ctx.enter_context(tc.tile_pool(name="const", bufs=1))
        big = ctx.enter_context(tc.tile_pool(name="big", bufs=1))
        tps = ctx.enter_context(tc.tile_pool(name="tps", bufs=2, space="PSUM"))
        mmp = ctx.enter_context(tc.tile_pool(name="mmp", bufs=2, space="PSUM"))
        ops = ctx.enter_context(tc.tile_pool(name="ops", bufs=2, space="PSUM"))
        ldp = ctx.enter_context(tc.tile_pool(name="ldp", bufs=3))
        wk = ctx.enter_context(tc.tile_pool(name="wk", bufs=2))

        # ---- constants (all pre-transposed host-side) ------------------
        eye = const.tile([128, 128], FP32, tag="eye")
        nc.sync.dma_start(eye[:, :], eye_d[:, :])

        wxpT = [const.tile([128, PROJ], FP32, name=f"wxpT{h}", tag=f"wxpT{h}")
                for h in range(2)]
        wxbT = [const.tile([128, R], FP32, name=f"wxbT{h}", tag=f"wxbT{h}")
                for h in range(2)]
        woutT = [const.tile([128, D], BF16, name=f"woutT{h}", tag=f"woutT{h}")
                 for h in range(2)]
        aneg = [const.tile([128, N], FP32, name=f"aneg{h}", tag=f"aneg{h}")
                for h in range(2)]
        bdt = [const.tile([128, 1], FP32, name=f"bdt{h}", tag=f"bdt{h}")
               for h in range(2)]
        dskip = [const.tile([128, 1], FP32, name=f"dsk{h}", tag=f"dsk{h}")
                 for h in range(2)]
        for h in range(2):
            hs = slice(h * 128, (h + 1) * 128)
            nc.sync.dma_start(wxpT[h][:, :], wxpT_d[hs, :])
            nc.sync.dma_start(wxbT[h][:, :], wxbT_d[hs, :])
            nc.sync.dma_start(woutT[h][:, :], woutT_d[hs, :])
            nc.sync.dma_start(aneg[h][:, :], aneg_d[hs, :])
            nc.sync.dma_start(bdt[h][:, :], bdt_d[hs, :])
            nc.sync.dma_start(dskip[h][:, :], dskip_d[hs, :])
        wdtT = const.tile([R, D], FP32, tag="wdtT")
        nc.sync.dma_start(wdtT[:, :], wdtT_d[:, :])

        # pre-touch DMA'd weights on PE so later matmuls don't accumulate
        # more sync-wait commands than the ISA allows
        warm = tps.tile([128, 128], FP32, tag="tps")
        nc.tensor.transpose(warm[:, :], eye[:, :], eye[:, :])
        warm2 = tps.tile([PROJ, 128], FP32, tag="tps")
        nc.tensor.matmul(warm2[:, :], wxpT[0][:, :], eye[:, :],
                         start=True, stop=True)

        # ---- x^T ------------------------------------------------------
        xT = [big.tile([128, L], FP32, name=f"xT{h}", tag=f"xT{h}") for h in range(2)]
        for i in range(L // 128):
            xn = ldp.tile([128, D], FP32, tag="ld256")
            nc.sync.dma_start(xn[:, :], x_d[i * 128:(i + 1) * 128, :])
            for h in range(2):
                pt = tps.tile([128, 128], FP32, tag="tps")
                nc.tensor.transpose(pt[:, :], xn[:, h * 128:(h + 1) * 128], eye[:, :])
                nc.scalar.copy(xT[h][:, i * 128:(i + 1) * 128], pt[:, :])

        # ---- projections + delta path (per LC chunk) -------------------
        # xdbl16 holds all 64 x_dbl rows in bf16; rows R.. feed the gathers.
        xdbl16 = big.tile([PROJ, L], BF16, tag="xdbl16")
        dT = [big.tile([128, L], BF16, name=f"dT{h}", tag=f"dT{h}") for h in range(2)]
        ubT = [big.tile([128, L], BF16, name=f"ubT{h}", tag=f"ubT{h}")
               for h in range(2)]
        xsk = [big.tile([128, L], BF16, name=f"xsk{h}", tag=f"xsk{h}")
               for h in range(2)]

        for c in range(NLC):
            sl = slice(c * LC, (c + 1) * LC)
            # x_dbl^T chunk (64, LC) = Wxp @ x^T
            pd = mmp.tile([128, LC], FP32, tag="mmp", bufs=3)
            for h in range(2):
                nc.tensor.matmul(pd[0:PROJ, :], wxpT[h][:, :], xT[h][:, sl],
                                 start=(h == 0), stop=(h == 1))
            # fp32 delta_r rows for the dt matmul; bf16 copy of everything
            drc = wk.tile([R, LC], FP32, tag="drc", bufs=1)
            nc.scalar.copy(drc[:, :], pd[0:R, :])
            nc.scalar.copy(xdbl16[:, sl], pd[0:PROJ, :])
            # xb^T chunk (16, LC) = W_xbproj @ x^T  (FORWARD order)
            pb = mmp.tile([128, LC], FP32, tag="mmp", bufs=3)
            for h in range(2):
                nc.tensor.matmul(pb[0:R, :], wxbT[h][:, :], xT[h][:, sl],
                                 start=(h == 0), stop=(h == 1))
            xbc = wk.tile([R, LC], FP32, tag="xbc", bufs=1)
            nc.scalar.copy(xbc[:, :], pb[0:R, :])
            for h in range(2):
                hsl = slice(h * 128, (h + 1) * 128)
                # delta = ln(exp(z + b_dt) + 1)  [softplus, one act table]
                pz = mmp.tile([128, LC], FP32, tag="mmp", bufs=3)
                nc.tensor.matmul(pz[:, :], wdtT[:, hsl], drc[:, :],
                                 start=True, stop=True)
                ez = wk.tile([128, LC], FP32, tag="ez")
                nc.scalar.activation(ez[:, :], pz[:, :], AF.Exp,
                                     bias=bdt[h][:, 0:1])
                nc.scalar.activation(dT[h][:, sl], ez[:, :], AF.Ln, bias=1.0)
                pz2 = mmp.tile([128, LC], FP32, tag="mmp", bufs=3)
                nc.tensor.matmul(pz2[:, :], wdtT[:, hsl], xbc[:, :],
                                 start=True, stop=True)
                ez2 = wk.tile([128, LC], FP32, tag="ez")
                nc.scalar.activation(ez2[:, :], pz2[:, :], AF.Exp,
                                     bias=bdt[h][:, 0:1])
                dbc = wk.tile([128, LC], BF16, tag="dbc")
                nc.scalar.activation(dbc[:, :], ez2[:, :], AF.Ln, bias=1.0)
                # ub = delta_b * x (forward order; read reversed later)
                nc.vector.tensor_mul(ubT[h][:, sl], dbc[:, :], xT[h][:, sl])
                # skip term (x + xf) * D_skip -> bf16 (matmul lhsT later)
                xs = wk.tile([128, LC], FP32, tag="ez")
                rsl = slice(L - (c + 1) * LC, L - c * LC)
                nc.vector.tensor_add(xs[:, :], xT[h][:, sl],
                                     _rev_ap(xT[h][:, rsl]))
                nc.scalar.activation(xsk[h][:, sl], xs[:, :], AF.Copy,
                                     scale=dskip[h][:, 0:1])

        # ---- main scan loop ------------------------------------------
        carry = [[None, None], [None, None]]    # [g][h] -> carry cols tile
        u_cur = [None, None]                    # per-h u chunk for this c
        for c in range(NLC):
            sl = slice(c * LC, (c + 1) * LC)
            rsl = slice(L - (c + 1) * LC, L - c * LC)
            tree = [[None, None], [None, None]]  # [g][h] -> y-part tile
            for g in range(G):
                n0 = g * NG
                # B/C rows for this n-group, replicated to 128 partitions
                bf_rep = wk.tile([128, NG * LC], BF16, tag="bfr")
                bb_rep = wk.tile([128, NG * LC], BF16, tag="bbr")
                c_rep = wk.tile([128, NG * LC], BF16, tag="ccr")
                bf_fl = wk.tile([1, NG * LC], BF16, tag="fl", bufs=2)
                bb_fl = wk.tile([1, NG * LC], BF16, tag="fl", bufs=2)
                c_fl = wk.tile([1, NG * LC], BF16, tag="fl", bufs=2)
                nc.sync.dma_start(_blk_ap(bf_fl[0:1, :], NG, LC),
                                  xdbl16[R + n0:R + n0 + NG, sl])
                nc.sync.dma_start(_blk_ap(bb_fl[0:1, :], NG, LC),
                                  xdbl16[R + N + n0:R + N + n0 + NG, sl])
                nc.sync.dma_start(_blk_ap(c_fl[0:1, :], NG, LC),
                                  xdbl16[R + 2 * N + n0:R + 2 * N + n0 + NG, sl])
                for rep, fl in ((bf_rep, bf_fl), (bb_rep, bb_fl), (c_rep, c_fl)):
                    s = fl[0:1, :]
                    src_b = AP(s.tensor, s.offset,
                               [[s.ap[0][0], 1], [0, 128], [1, NG * LC]])
                    nc.sync.dma_start(rep[:, :], src_b)
                for h in range(2):
                    if g == 0:
                        ut = wk.tile([128, LC], BF16, tag=f"ut{h}", bufs=2)
                        nc.vector.tensor_mul(ut[:, :], dT[h][:, sl],
                                             xT[h][:, sl])
                        u_cur[h] = ut
                    a_t = wk.tile([128, NG * LC], BF16, tag="at")
                    for j in range(NG):
                        n = n0 + j
                        nc.scalar.activation(a_t[:, j * LC:(j + 1) * LC],
                                             dT[h][:, sl], AF.Exp,
                                             scale=aneg[h][:, n:n + 1])
                    # ptm doubles as p-product scratch and later h*C tree buf
                    ptm = wk.tile([128, NG * LC], BF16, tag="tm")
                    b_t = wk.tile([128, NG * LC], BF16, tag="bt")
                    nc.vector.tensor_tensor(_blk_ap(ptm[:, :], NG, LC),
                                            _rep_ap(u_cur[h][:, :], NG),
                                            _blk_ap(bf_rep[:, :], NG, LC), ALU.mult)
                    nc.vector.tensor_tensor(_blk_ap(b_t[:, :], NG, LC),
                                            _rep_rev_ap(ubT[h][:, rsl], NG),
                                            _blk_ap(bb_rep[:, :], NG, LC), ALU.mult)
                    nc.vector.tensor_add(b_t[:, :], b_t[:, :], ptm[:, :])
                    h_t = wk.tile([128, NG * LC], BF16, tag="ht", bufs=2)
                    seng = nc.gpsimd if (c, g, h) in SCAN_POOL else nc.vector
                    for j in range(NG):
                        js = slice(j * LC, (j + 1) * LC)
                        if c == 0:
                            init = 0.0
                        else:
                            init = carry[g][h][:, j:j + 1]
                        seng.tensor_tensor_scan(h_t[:, js], a_t[:, js],
                                                b_t[:, js], init,
                                                ALU.mult, ALU.add)
                    if c < NLC - 1:
                        cy = wk.tile([128, NG], BF16, tag=f"cy{g}{h}", bufs=2)
                        nc.vector.tensor_copy(
                            cy[:, :], AP(h_t.tensor, h_t[:, :].offset + LC - 1,
                                         [[h_t[:, :].ap[0][0], 128], [LC, NG]]))
                        carry[g][h] = cy
                    # y-part for this group: tree-reduce h*C over the 8 n's
                    teng = nc.gpsimd if (c, g, h) in TREE_POOL else nc.vector
                    tmp = ptm
                    teng.tensor_mul(tmp[:, :], h_t[:, :], c_rep[:, :])
                    half = NG * LC // 2
                    while half >= 2 * LC:
                        teng.tensor_add(tmp[:, 0:half], tmp[:, 0:half],
                                        tmp[:, half:2 * half])
                        half //= 2
                    yg = wk.tile([128, LC], BF16, tag=f"yg{g}{h}", bufs=2)
                    teng.tensor_add(yg[:, :], tmp[:, 0:LC], tmp[:, LC:2 * LC])
                    tree[g][h] = yg
            # out projection: psum accumulates (yg0 + yg1 + xsk) @ W_out^T
            for s in range(LC // LSUB):
                l0 = c * LC + s * LSUB
                ssl = slice(s * LSUB, (s + 1) * LSUB)
                pt = ops.tile([LSUB, D], FP32, tag="ops")
                terms = []
                for h in range(2):
                    terms += [(tree[0][h][:, ssl], h), (tree[1][h][:, ssl], h),
                              (xsk[h][:, l0:l0 + LSUB], h)]
                for k, (term, h) in enumerate(terms):
                    nc.tensor.matmul(pt[:, :], term, woutT[h][:, :],
                                     start=(k == 0), stop=(k == len(terms) - 1))
                ot = wk.tile([LSUB, D], FP32, tag="osb")
                nc.scalar.copy(ot[:, :], pt[:, :])
                nc.sync.dma_start(out_d[l0:l0 + LSUB, :], ot[:, :])


_NC_CACHE = {}  # v3


def _build():
    if "nc" in _NC_CACHE:
        return _NC_CACHE["nc"]
    nc = bacc.Bacc("TRN2", target_bir_lowering=False, debug=False,
                   num_devices=NCORES)
    x_d = nc.dram_tensor("x", [L, D], FP32, kind="ExternalInput").ap()
    wxpT_d = nc.dram_tensor("WxpT", [D, PROJ], FP32, kind="ExternalInput").ap()
    wxbT_d = nc.dram_tensor("WxbT", [D, R], FP32, kind="ExternalInput").ap()
    wdtT_d = nc.dram_tensor("WdtT", [R, D], FP32, kind="ExternalInput").ap()
    bdt_d = nc.dram_tensor("bdt", [D, 1], FP32, kind="ExternalInput").ap()
    aneg_d = nc.dram_tensor("Aneg", [D, N], FP32, kind="ExternalInput").ap()
    dskip_d = nc.dram_tensor("Dskip", [D, 1], FP32, kind="ExternalInput").ap()
    woutT_d = nc.dram_tensor("WoutT", [D, D], BF16, kind="ExternalInput").ap()
    eye_d = nc.dram_tensor("eye", [128, 128], FP32, kind="ExternalInput").ap()
    out_d = nc.dram_tensor("out", [L, D], FP32, kind="ExternalOutput").ap()
    io = (x_d, wxpT_d, wxbT_d, wdtT_d, bdt_d, aneg_d, dskip_d, woutT_d,
          eye_d, out_d)
    with tile.TileContext(nc) as tc:
        _emit(tc, nc, io)
    nc.compile()
    _NC_CACHE["nc"] = nc
    return nc


def host_prep(W_xproj, W_xbproj, W_dt, b_dt, A_log, D_skip, W_out):
    """Host-side input transforms shared by all cores."""
    import ml_dtypes

    return {
        "WxpT": np.ascontiguousarray(np.asarray(W_xproj, dtype=np.float32).T),
        "WxbT": np.ascontiguousarray(np.asarray(W_xbproj, dtype=np.float32).T),
        "WdtT": np.ascontiguousarray(np.asarray(W_dt, dtype=np.float32).T),
        "bdt": np.ascontiguousarray(
            np.asarray(b_dt, dtype=np.float32).reshape(D, 1)),
        "Aneg": np.ascontiguousarray(
            -np.exp(np.asarray(A_log, dtype=np.float32))),
        "Dskip": np.ascontiguousarray(
            np.asarray(D_skip, dtype=np.float32).reshape(D, 1)),
        "WoutT": np.ascontiguousarray(
            np.asarray(W_out, dtype=np.float32).T.astype(ml_dtypes.bfloat16)),
        "eye": np.eye(128, dtype=np.float32),
    }


def kernel(x, W_xproj, W_xbproj, W_dt, b_dt, A_log, D_skip, W_out, **profile_kw):
    nc = _build()
    shared = host_prep(W_xproj, W_xbproj, W_dt, b_dt, A_log, D_skip, W_out)
    xs = np.asarray(x, dtype=np.float32)
    in_maps = [{"x": np.ascontiguousarray(xs[b]), **shared} for b in range(NCORES)]
    res = bass_utils.run_bass_kernel_spmd(nc, in_maps, core_ids=list(range(NCORES)),
                                          **profile_kw)
    out = np.stack([res.results[b]["out"] for b in range(NCORES)], axis=0)
    kernel.last_result = res
    return out


# revision 18
# speedup vs baseline: 1.6259x; 1.1205x over previous
"""Trainium2 Bass kernel for a bidirectional selective-scan SSM (Mamba-like).

Problem: nn_ProMU_42623255445559
  B=8, L=2048, D=256, N=16, R=16
  Data-parallel over batch: core i handles batch row i; weights replicated.

v3 dataflow (d on partitions, l in free; two 128-partition halves):
  x_dbl^T = Wxp @ x^T                  (PE)
  delta   = softplus(Wdt @ delta_r^T + b_dt) = ln(exp(z)+1)   (ACT exp+ln,
            single act-func table: ln/exp/copy/identity share set 6)
  delta_b computed in FORWARD order from x (not xf); consumers read it with
            reversed APs, so xf^T is never materialized.
  a_n     = exp(A_n * delta)           (ACT, per-partition scale = A_n < 0)
  b_n     = u*Bf_n + ub_rev*Bb_n       (DVE bf16 2x; u=delta*x, ub=delta_b*x)
  h_n     = scan(a, b) along l         (Pool engine; DVE stays on mults)
  yg      = tree-reduce_n (h_n * C_n)  (DVE bf16 2x, per n-group of 8)
  out     = (yg0 + yg1 + (x+xf)*D_skip) @ W_out^T
            -- assembled in PSUM: 6 accumulating bf16 matmuls (PE)

Host-side prep: weight transposes, A=-exp(A_log), +b_dt, bf16 W_out.
"""

import sys

sys.path.insert(0, "/opt/trn_rl_repo")

from contextlib import ExitStack

import numpy as np

import concourse.bacc as bacc
import concourse.bass as bass
import concourse.mybir as mybir
import concourse.tile as tile
from concourse import bass_utils
from concourse.bass import AP

B, L, D, N, R = 8, 2048, 256, 16, 16
PROJ = R + 3 * N  # 64 rows of x_dbl^T
FP32 = mybir.dt.float32
BF16 = mybir.dt.bfloat16
AF = mybir.ActivationFunctionType
ALU = mybir.AluOpType

NCORES = 8
LC = 512          # l-chunk for the scan pipeline
NLC = L // LC     # 4
NG = 8            # n per group
G = N // NG       # 2 groups
LSUB = 128        # l-subchunk for out-proj matmuls

# which (c, g, h) iterations run their reduce tree on Pool (balance tuning)
TREE_POOL = {(c, g, h) for c in range(NLC) for g in range(G) for h in range(2)}
# scans are DVE-only (TPB ISA rejects the scan opcode on Pool)
SCAN_POOL = set()


def _rev_ap(ap2d):
    """Reverse the (single) free dim of a [P, F] AP."""
    (pstep, pcount), (fstep, fcount) = ap2d.ap
    assert fstep == 1
    return AP(ap2d.tensor, ap2d.offset + fcount - 1, [[pstep, pcount], [-1, fcount]])


def _rep_ap(ap2d, r):
    """Repeat a [P, F] AP r times along free -> [P, r, F] with stride 0."""
    (pstep, pcount), (fstep, fcount) = ap2d.ap
    assert fstep == 1
    return AP(ap2d.tensor, ap2d.offset, [[pstep, pcount], [0, r], [1, fcount]])


def _rep_rev_ap(ap2d, r):
    """Repeat the REVERSED [P, F] AP r times along free -> [P, r, F]."""
    (pstep, pcount), (fstep, fcount) = ap2d.ap
    assert fstep == 1
    return AP(ap2d.tensor, ap2d.offset + fcount - 1,
              [[pstep, pcount], [0, r], [-1, fcount]])


def _blk_ap(ap2d, r, f):
    """View a [P, r*f] AP as [P, r, f]."""
    (pstep, pcount), (fstep, fcount) = ap2d.ap
    assert fstep == 1 and fcount == r * f
    return AP(ap2d.tensor, ap2d.offset, [[pstep, pcount], [f, r], [1, f]])


def _emit(tc, nc, io):
    x_d, wxpT_d, wxbT_d, wdtT_d, bdt_d, aneg_d, dskip_d, woutT_d, eye_d, out_d = io

    ctx = ExitStack()
    with ctx:
        const = ctx.enter_context(tc.tile_pool(name="const", bufs=1))
        big = ctx.enter_context(tc.tile_pool(name="big", bufs=1))
        tps = ctx.enter_context(tc.tile_pool(name="tps", bufs=2, space="PSUM"))
        mmp = ctx.enter_context(tc.tile_pool(name="mmp", bufs=2, space="PSUM"))
        ops = ctx.enter_context(tc.tile_pool(name="ops", bufs=2, space="PSUM"))
        ldp = ctx.enter_context(tc.tile_pool(name="ldp", bufs=3))
        wk = ctx.enter_context(tc.tile_pool(name="wk", bufs=2))
        drp = ctx.enter_context(tc.tile_pool(name="drp", bufs=1, space="DRAM"))

        # ---- constants (all pre-transposed host-side) ------------------
        eye = const.tile([128, 128], FP32, tag="eye")
        nc.sync.dma_start(eye[:, :], eye_d[:, :])
        # x loads issued before the other consts (they gate the prologue)
        xldp = []
        for cq in range(4):
            xn = ldp.tile([128, 4 * D], FP32, tag="ld4", bufs=2)
            s = x_d[cq * 512:cq * 512 + 128, :]
            src4 = AP(s.tensor, s.offset,
                      [[s.ap[0][0], 128], [128 * s.ap[0][0], 4], [1, D]])
            dst4 = AP(xn.tensor, xn[:, :].offset,
                      [[xn[:, :].ap[0][0], 128], [D, 4], [1, D]])
            nc.sync.dma_start(dst4, src4)
            xldp.append(xn)

        wxpT = [const.tile([128, PROJ], BF16, name=f"wxpT{h}", tag=f"wxpT{h}")
                for h in range(2)]
        wxbT = [const.tile([128, R], BF16, name=f"wxbT{h}", tag=f"wxbT{h}")
                for h in range(2)]
        woutT = [const.tile([128, D], BF16, name=f"woutT{h}", tag=f"woutT{h}")
                 for h in range(2)]
        aneg = [const.tile([128, N], FP32, name=f"aneg{h}", tag=f"aneg{h}")
                for h in range(2)]
        bdt = [const.tile([128, 1], FP32, name=f"bdt{h}", tag=f"bdt{h}")
               for h in range(2)]
        dskip = [const.tile([128, 1], FP32, name=f"dsk{h}", tag=f"dsk{h}")
                 for h in range(2)]
        for h in range(2):
            hs = slice(h * 128, (h + 1) * 128)
            nc.scalar.dma_start(wxpT[h][:, :], wxpT_d[hs, :])
            nc.scalar.dma_start(wxbT[h][:, :], wxbT_d[hs, :])
            nc.scalar.dma_start(woutT[h][:, :], woutT_d[hs, :])
            nc.scalar.dma_start(aneg[h][:, :], aneg_d[hs, :])
            nc.scalar.dma_start(bdt[h][:, :], bdt_d[hs, :])
            nc.scalar.dma_start(dskip[h][:, :], dskip_d[hs, :])
        wdtT = const.tile([R, D], BF16, tag="wdtT")
        nc.scalar.dma_start(wdtT[:, :], wdtT_d[:, :])

        # pre-touch DMA'd weights on PE so later matmuls don't accumulate
        # more sync-wait commands than the ISA allows
        warm = tps.tile([128, 128], FP32, tag="tps")
        nc.tensor.transpose(warm[:, :], eye[:, :], eye[:, :])
        warm2 = tps.tile([128, 128], FP32, tag="tps")
        nc.tensor.matmul(warm2[:, :], eye[:, :], eye[:, :],
                         start=True, stop=True)

        # ---- x^T ------------------------------------------------------
        xT = [big.tile([128, L], BF16, name=f"xT{h}", tag=f"xT{h}") for h in range(2)]
        for cq in range(4):
            xn = xldp[cq]
            for i4 in range(4):
                i = cq * 4 + i4
                for h in range(2):
                    pt = tps.tile([128, 128], FP32, tag="tps")
                    nc.tensor.transpose(pt[:, :],
                                        xn[:, i4 * D + h * 128:i4 * D + (h + 1) * 128],
                                        eye[:, :])
                    nc.scalar.copy(xT[h][:, i * 128:(i + 1) * 128],
                                   pt[:, :])

        # ---- projections + delta path (per LC chunk) -------------------
        # B/C rows of x_dbl (bf16) staged in DRAM; broadcasts read from there.
        # exp/ln phases are batched so the ACT engine never swaps func tables
        # (Exp lives in set 0, Ln in set 5, Copy in every set).
        xdbd = drp.tile([3 * N, L], BF16, tag="xdbd")
        zf = [big.tile([128, L], BF16, name=f"zf{h}", tag=f"zf{h}")
              for h in range(2)]
        zb = [big.tile([128, L], BF16, name=f"zb{h}", tag=f"zb{h}")
              for h in range(2)]
        dT = [big.tile([128, L], BF16, name=f"dT{h}", tag=f"dT{h}") for h in range(2)]
        ubT = [big.tile([128, L], BF16, name=f"ubT{h}", tag=f"ubT{h}")
               for h in range(2)]
        xsk = [big.tile([128, L], BF16, name=f"xsk{h}", tag=f"xsk{h}")
               for h in range(2)]

        for c in range(NLC):
            sl = slice(c * LC, (c + 1) * LC)
            # x_dbl^T chunk (64, LC) = Wxp @ x^T
            pd = mmp.tile([128, LC], FP32, tag="mmp", bufs=3)
            for h in range(2):
                nc.tensor.matmul(pd[0:PROJ, :], wxpT[h][:, :], xT[h][:, sl],
                                 start=(h == 0), stop=(h == 1))
            # fp32 delta_r rows for the dt matmul; bf16 B/C rows -> DRAM
            drc = wk.tile([R, LC], BF16, tag="drc", bufs=1)
            nc.scalar.copy(drc[:, :], pd[0:R, :])
            bcc = wk.tile([PROJ, LC], BF16, tag="bcc")
            nc.scalar.copy(bcc[:, :], pd[0:PROJ, :])
            nc.sync.dma_start(xdbd[:, sl], bcc[R:PROJ, :])
            # xb^T chunk (16, LC) = W_xbproj @ x^T  (FORWARD order)
            pb = mmp.tile([128, LC], FP32, tag="mmp", bufs=3)
            for h in range(2):
                nc.tensor.matmul(pb[0:R, :], wxbT[h][:, :], xT[h][:, sl],
                                 start=(h == 0), stop=(h == 1))
            xbc = wk.tile([R, LC], BF16, tag="xbc", bufs=1)
            nc.scalar.copy(xbc[:, :], pb[0:R, :])
            for h in range(2):
                hsl = slice(h * 128, (h + 1) * 128)
                # z = W_dt @ delta_r^T (+b_dt later); staged to SBUF by Pool
                pz = mmp.tile([128, LC], FP32, tag="mmp", bufs=3)
                nc.tensor.matmul(pz[:, :], wdtT[:, hsl], drc[:, :],
                                 start=True, stop=True)
                nc.scalar.copy(zf[h][:, sl], pz[:, :])
                pz2 = mmp.tile([128, LC], FP32, tag="mmp", bufs=3)
                nc.tensor.matmul(pz2[:, :], wdtT[:, hsl], xbc[:, :],
                                 start=True, stop=True)
                nc.scalar.copy(zb[h][:, sl], pz2[:, :])

        # delta = ln(exp(z + b_dt) + 1) [softplus]: full-L exp then ln per
        # direction-half -- 8 ACT instrs, no act-table swaps mid-stream
        dbT = [big.tile([128, L], BF16, name=f"dbT{h}", tag=f"dbT{h}")
               for h in range(2)]
        for h in range(2):
            nc.scalar.activation(zf[h][:, :], zf[h][:, :], AF.Exp,
                                 bias=bdt[h][:, 0:1])
            nc.scalar.activation(zb[h][:, :], zb[h][:, :], AF.Exp,
                                 bias=bdt[h][:, 0:1])
        for h in range(2):
            nc.scalar.activation(dT[h][:, :], zf[h][:, :], AF.Ln, bias=1.0)
            nc.scalar.activation(dbT[h][:, :], zb[h][:, :], AF.Ln, bias=1.0)
        for c in range(NLC):
            sl = slice(c * LC, (c + 1) * LC)
            rsl = slice(L - (c + 1) * LC, L - c * LC)
            for h in range(2):
                # ub = delta_b * x (forward order; read reversed later)
                nc.gpsimd.tensor_mul(ubT[h][:, sl], dbT[h][:, sl],
                                     xT[h][:, sl])
                # skip term (x + xf) * D_skip -> bf16 (matmul lhsT later)
                xs = wk.tile([128, LC], BF16, tag="ez")
                nc.gpsimd.tensor_add(xs[:, :], xT[h][:, sl],
                                     _rev_ap(xT[h][:, rsl]))
                nc.scalar.activation(xsk[h][:, sl], xs[:, :], AF.Copy,
                                     scale=dskip[h][:, 0:1])

        # ---- main scan loop ------------------------------------------
        carry = [[None, None], [None, None]]    # [g][h] -> carry cols tile
        u_cur = [None, None]                    # per-h u chunk for this c
        for c in range(NLC):
            sl = slice(c * LC, (c + 1) * LC)
            rsl = slice(L - (c + 1) * LC, L - c * LC)
            tree = [[None, None], [None, None]]  # [g][h] -> y-part tile
            reps = []
            for g in range(G):
                n0 = g * NG
                # B/C rows for this n-group, replicated to 128 partitions
                bf_rep = wk.tile([128, NG * LC], BF16, tag="bfr")
                bb_rep = wk.tile([128, NG * LC], BF16, tag="bbr")
                c_rep = wk.tile([128, NG * LC], BF16, tag="ccr")
                for rep, r0, qeng in ((bf_rep, n0, nc.sync),
                                      (bb_rep, N + n0, nc.sync),
                                      (c_rep, 2 * N + n0, nc.scalar)):
                    s = xdbd[r0:r0 + NG, sl]
                    src_b = AP(s.tensor, s.offset,
                               [[0, 128], [s.ap[0][0], NG], [1, LC]])
                    qeng.dma_start(_blk_ap(rep[:, :], NG, LC), src_b)
                reps.append((bf_rep, bb_rep, c_rep))
            for g in range(G):
                n0 = g * NG
                bf_rep, bb_rep, c_rep = reps[g]
                for h in range(2):
                    if g == 0:
                        ut = wk.tile([128, LC], BF16, tag=f"ut{h}", bufs=2)
                        nc.vector.tensor_mul(ut[:, :], dT[h][:, sl],
                                             xT[h][:, sl])
                        u_cur[h] = ut
                    a_t = wk.tile([128, NG * LC], BF16, tag="at")
                    for j in range(NG):
                        n = n0 + j
                        nc.scalar.activation(a_t[:, j * LC:(j + 1) * LC],
                                             dT[h][:, sl], AF.Exp,
                                             scale=aneg[h][:, n:n + 1])
                    # ptm doubles as p-product scratch and later h*C tree buf
                    ptm = wk.tile([128, NG * LC], BF16, tag="tm")
                    b_t = wk.tile([128, NG * LC], BF16, tag="bt")
                    nc.vector.tensor_tensor(_blk_ap(ptm[:, :], NG, LC),
                                            _rep_ap(u_cur[h][:, :], NG),
                                            _blk_ap(bf_rep[:, :], NG, LC), ALU.mult)
                    nc.vector.tensor_tensor(_blk_ap(b_t[:, :], NG, LC),
                                            _rep_rev_ap(ubT[h][:, rsl], NG),
                                            _blk_ap(bb_rep[:, :], NG, LC), ALU.mult)
                    badd_eng = (nc.vector if (g, h) == (1, 1)
                                else nc.gpsimd)
                    badd_eng.tensor_add(b_t[:, :], b_t[:, :], ptm[:, :])
                    h_t = wk.tile([128, NG * LC], BF16, tag="ht", bufs=2)
                    seng = nc.gpsimd if (c, g, h) in SCAN_POOL else nc.vector
                    for j in range(NG):
                        js = slice(j * LC, (j + 1) * LC)
                        if c == 0:
                            init = 0.0
                        else:
                            init = carry[g][h][:, j:j + 1]
                        seng.tensor_tensor_scan(h_t[:, js], a_t[:, js],
                                                b_t[:, js], init,
                                                ALU.mult, ALU.add)
                    if c < NLC - 1:
                        cy = wk.tile([128, NG], BF16, tag=f"cy{g}{h}", bufs=2)
                        nc.gpsimd.tensor_copy(
                            cy[:, :], AP(h_t.tensor, h_t[:, :].offset + LC - 1,
                                         [[h_t[:, :].ap[0][0], 128], [LC, NG]]))
                        carry[g][h] = cy
                    # y-part for this group: tree-reduce h*C over the 8 n's
                    teng = nc.gpsimd if (c, g, h) in TREE_POOL else nc.vector
                    tmp = ptm
                    teng.tensor_mul(tmp[:, :], h_t[:, :], c_rep[:, :])
                    half = NG * LC // 2
                    while half >= 2 * LC:
                        teng.tensor_add(tmp[:, 0:half], tmp[:, 0:half],
                                        tmp[:, half:2 * half])
                        half //= 2
                    yg = wk.tile([128, LC], BF16, tag=f"yg{g}{h}", bufs=2)
                    teng.tensor_add(yg[:, :], tmp[:, 0:LC], tmp[:, LC:2 * LC])
                    tree[g][h] = yg
            # out projection: psum accumulates (yg0 + yg1 + xsk) @ W_out^T
            for s in range(LC // LSUB):
                l0 = c * LC + s * LSUB
                ssl = slice(s * LSUB, (s + 1) * LSUB)
                pt = ops.tile([LSUB, D], FP32, tag="ops")
                terms = []
                for h in range(2):
                    terms += [(tree[0][h][:, ssl], h), (tree[1][h][:, ssl], h),
                              (xsk[h][:, l0:l0 + LSUB], h)]
                for k, (term, h) in enumerate(terms):
                    nc.tensor.matmul(pt[:, :], term, woutT[h][:, :],
                                     start=(k == 0), stop=(k == len(terms) - 1))
                ot = wk.tile([LSUB, D], FP32, tag="osb")
                nc.scalar.copy(ot[:, :], pt[:, :])
                nc.sync.dma_start(out_d[l0:l0 + LSUB, :], ot[:, :])


_NC_CACHE = {}  # v3


def _build():
    if "nc" in _NC_CACHE:
        return _NC_CACHE["nc"]
    nc = bacc.Bacc("TRN2", target_bir_lowering=False, debug=False,
                   num_devices=NCORES)
    x_d = nc.dram_tensor("x", [L, D], FP32, kind="ExternalInput").ap()
    wxpT_d = nc.dram_tensor("WxpT", [D, PROJ], BF16, kind="ExternalInput").ap()
    wxbT_d = nc.dram_tensor("WxbT", [D, R], BF16, kind="ExternalInput").ap()
    wdtT_d = nc.dram_tensor("WdtT", [R, D], BF16, kind="ExternalInput").ap()
    bdt_d = nc.dram_tensor("bdt", [D, 1], FP32, kind="ExternalInput").ap()
    aneg_d = nc.dram_tensor("Aneg", [D, N], FP32, kind="ExternalInput").ap()
    dskip_d = nc.dram_tensor("Dskip", [D, 1], FP32, kind="ExternalInput").ap()
    woutT_d = nc.dram_tensor("WoutT", [D, D], BF16, kind="ExternalInput").ap()
    eye_d = nc.dram_tensor("eye", [128, 128], FP32, kind="ExternalInput").ap()
    out_d = nc.dram_tensor("out", [L, D], FP32, kind="ExternalOutput").ap()
    io = (x_d, wxpT_d, wxbT_d, wdtT_d, bdt_d, aneg_d, dskip_d, woutT_d,
          eye_d, out_d)
    with tile.TileContext(nc) as tc:
        _emit(tc, nc, io)
    nc.compile()
    _NC_CACHE["nc"] = nc
    return nc


def host_prep(W_xproj, W_xbproj, W_dt, b_dt, A_log, D_skip, W_out):
    """Host-side input transforms shared by all cores."""
    import ml_dtypes

    return {
        "WxpT": np.ascontiguousarray(
            np.asarray(W_xproj, dtype=np.float32).T.astype(ml_dtypes.bfloat16)),
        "WxbT": np.ascontiguousarray(
            np.asarray(W_xbproj, dtype=np.float32).T.astype(ml_dtypes.bfloat16)),
        "WdtT": np.ascontiguousarray(
            np.asarray(W_dt, dtype=np.float32).T.astype(ml_dtypes.bfloat16)),
        "bdt": np.ascontiguousarray(
            np.asarray(b_dt, dtype=np.float32).reshape(D, 1)),
        "Aneg": np.ascontiguousarray(
            -np.exp(np.asarray(A_log, dtype=np.float32))),
        "Dskip": np.ascontiguousarray(
            np.asarray(D_skip, dtype=np.float32).reshape(D, 1)),
        "WoutT": np.ascontiguousarray(
            np.asarray(W_out, dtype=np.float32).T.astype(ml_dtypes.bfloat16)),
        "eye": np.eye(128, dtype=np.float32),
    }


def kernel(x, W_xproj, W_xbproj, W_dt, b_dt, A_log, D_skip, W_out, **profile_kw):
    nc = _build()
    shared = host_prep(W_xproj, W_xbproj, W_dt, b_dt, A_log, D_skip, W_out)
    xs = np.asarray(x, dtype=np.float32)
    in_maps = [{"x": np.ascontiguousarray(xs[b]), **shared} for b in range(NCORES)]
    res = bass_utils.run_bass_kernel_spmd(nc, in_maps, core_ids=list(range(NCORES)),
                                          **profile_kw)
    out = np.stack([res.results[b]["out"] for b in range(NCORES)], axis=0)
    kernel.last_result = res
    return out


# revision 24
# speedup vs baseline: 1.9279x; 1.1857x over previous
"""Trainium2 Bass kernel for a bidirectional selective-scan SSM (Mamba-like).

Problem: nn_ProMU_42623255445559
  B=8, L=2048, D=256, N=16, R=16
  Data-parallel over batch: core i handles batch row i; weights replicated.

v3 dataflow (d on partitions, l in free; two 128-partition halves):
  x_dbl^T = Wxp @ x^T                  (PE)
  delta   = softplus(Wdt @ delta_r^T + b_dt) = ln(exp(z)+1)   (ACT exp+ln,
            single act-func table: ln/exp/copy/identity share set 6)
  delta_b computed in FORWARD order from x (not xf); consumers read it with
            reversed APs, so xf^T is never materialized.
  a_n     = exp(A_n * delta)           (ACT, per-partition scale = A_n < 0)
  b_n     = u*Bf_n + ub_rev*Bb_n       (DVE bf16 2x; u=delta*x, ub=delta_b*x)
  h_n     = scan(a, b) along l         (Pool engine; DVE stays on mults)
  yg      = tree-reduce_n (h_n * C_n)  (DVE bf16 2x, per n-group of 8)
  out     = (yg0 + yg1 + (x+xf)*D_skip) @ W_out^T
            -- assembled in PSUM: 6 accumulating bf16 matmuls (PE)

Host-side prep: weight transposes, A=-exp(A_log), +b_dt, bf16 W_out.
"""

import sys

sys.path.insert(0, "/opt/trn_rl_repo")

from contextlib import ExitStack

import numpy as np

import concourse.bacc as bacc
import concourse.bass as bass
import concourse.mybir as mybir
import concourse.tile as tile
from concourse import bass_utils
from concourse.bass import AP

B, L, D, N, R = 8, 2048, 256, 16, 16
PROJ = R + 3 * N  # 64 rows of x_dbl^T
FP32 = mybir.dt.float32
BF16 = mybir.dt.bfloat16
AF = mybir.ActivationFunctionType
ALU = mybir.AluOpType

NCORES = 8
LC = 512          # l-chunk for the scan pipeline
NLC = L // LC     # 4
NG = 8            # n per group
G = N // NG       # 2 groups
LSUB = 128        # l-subchunk for out-proj matmuls

# which (c, g, h) iterations run their reduce tree on Pool (balance tuning)
TREE_POOL = {(c, g, h) for c in range(NLC) for g in range(G) for h in range(2)}
# scans are DVE-only (TPB ISA rejects the scan opcode on Pool)
SCAN_POOL = set()


def _rev_ap(ap2d):
    """Reverse the (single) free dim of a [P, F] AP."""
    (pstep, pcount), (fstep, fcount) = ap2d.ap
    assert fstep == 1
    return AP(ap2d.tensor, ap2d.offset + fcount - 1, [[pstep, pcount], [-1, fcount]])


def _rep_ap(ap2d, r):
    """Repeat a [P, F] AP r times along free -> [P, r, F] with stride 0."""
    (pstep, pcount), (fstep, fcount) = ap2d.ap
    assert fstep == 1
    return AP(ap2d.tensor, ap2d.offset, [[pstep, pcount], [0, r], [1, fcount]])


def _rep_rev_ap(ap2d, r):
    """Repeat the REVERSED [P, F] AP r times along free -> [P, r, F]."""
    (pstep, pcount), (fstep, fcount) = ap2d.ap
    assert fstep == 1
    return AP(ap2d.tensor, ap2d.offset + fcount - 1,
              [[pstep, pcount], [0, r], [-1, fcount]])


def _blk_ap(ap2d, r, f):
    """View a [P, r*f] AP as [P, r, f]."""
    (pstep, pcount), (fstep, fcount) = ap2d.ap
    assert fstep == 1 and fcount == r * f
    return AP(ap2d.tensor, ap2d.offset, [[pstep, pcount], [f, r], [1, f]])


def _emit(tc, nc, io):
    x_d, wxpT_d, wxbT_d, wdtT_d, bdt_d, aneg_d, dskip_d, woutT_d, eye_d, out_d = io

    ctx = ExitStack()
    with ctx:
        const = ctx.enter_context(tc.tile_pool(name="const", bufs=1))
        big = ctx.enter_context(tc.tile_pool(name="big", bufs=1))
        tps = ctx.enter_context(tc.tile_pool(name="tps", bufs=2, space="PSUM"))
        mmp = ctx.enter_context(tc.tile_pool(name="mmp", bufs=2, space="PSUM"))
        ops = ctx.enter_context(tc.tile_pool(name="ops", bufs=2, space="PSUM"))
        ldp = ctx.enter_context(tc.tile_pool(name="ldp", bufs=3))
        wk = ctx.enter_context(tc.tile_pool(name="wk", bufs=2))
        drp = ctx.enter_context(tc.tile_pool(name="drp", bufs=1, space="DRAM"))

        # ---- constants (all pre-transposed host-side) ------------------
        eye = const.tile([128, 128], FP32, tag="eye")
        nc.sync.dma_start(eye[:, :], eye_d[:, :])
        # x loads issued before the other consts (they gate the prologue)
        xldp = []
        for cq in range(4):
            xn = ldp.tile([128, 4 * D], FP32, tag="ld4", bufs=2)
            s = x_d[cq * 512:cq * 512 + 128, :]
            src4 = AP(s.tensor, s.offset,
                      [[s.ap[0][0], 128], [128 * s.ap[0][0], 4], [1, D]])
            dst4 = AP(xn.tensor, xn[:, :].offset,
                      [[xn[:, :].ap[0][0], 128], [D, 4], [1, D]])
            nc.sync.dma_start(dst4, src4)
            xldp.append(xn)

        wxpT = [const.tile([128, PROJ], BF16, name=f"wxpT{h}", tag=f"wxpT{h}")
                for h in range(2)]
        wxbT = [const.tile([128, R], BF16, name=f"wxbT{h}", tag=f"wxbT{h}")
                for h in range(2)]
        woutT = [const.tile([128, D], BF16, name=f"woutT{h}", tag=f"woutT{h}")
                 for h in range(2)]
        aneg = [const.tile([128, N], FP32, name=f"aneg{h}", tag=f"aneg{h}")
                for h in range(2)]
        bdt = [const.tile([128, 1], FP32, name=f"bdt{h}", tag=f"bdt{h}")
               for h in range(2)]
        dskip = [const.tile([128, 1], FP32, name=f"dsk{h}", tag=f"dsk{h}")
                 for h in range(2)]
        for h in range(2):
            hs = slice(h * 128, (h + 1) * 128)
            nc.sync.dma_start(wxpT[h][:, :], wxpT_d[hs, :])
            nc.sync.dma_start(wxbT[h][:, :], wxbT_d[hs, :])
            nc.sync.dma_start(woutT[h][:, :], woutT_d[hs, :])
            nc.sync.dma_start(aneg[h][:, :], aneg_d[hs, :])
            nc.sync.dma_start(bdt[h][:, :], bdt_d[hs, :])
            nc.sync.dma_start(dskip[h][:, :], dskip_d[hs, :])
        wdtT = const.tile([R, D], BF16, tag="wdtT")
        nc.sync.dma_start(wdtT[:, :], wdtT_d[:, :])

        # pre-touch DMA'd weights on PE so later matmuls don't accumulate
        # more sync-wait commands than the ISA allows
        warm = tps.tile([128, 128], FP32, tag="tps")
        nc.tensor.transpose(warm[:, :], eye[:, :], eye[:, :])
        warm2 = tps.tile([128, 128], FP32, tag="tps")
        nc.tensor.matmul(warm2[:, :], eye[:, :], eye[:, :],
                         start=True, stop=True)

        # ---- x^T ------------------------------------------------------
        xT = [big.tile([128, L], BF16, name=f"xT{h}", tag=f"xT{h}") for h in range(2)]
        for cq in range(4):
            xn = xldp[cq]
            for i4 in range(4):
                i = cq * 4 + i4
                for h in range(2):
                    pt = tps.tile([128, 128], FP32, tag="tps")
                    nc.tensor.transpose(pt[:, :],
                                        xn[:, i4 * D + h * 128:i4 * D + (h + 1) * 128],
                                        eye[:, :])
                    nc.vector.tensor_copy(
                        xT[h][:, i * 128:(i + 1) * 128], pt[:, :])

        # ---- projections + delta path (per LC chunk) -------------------
        # B/C rows of x_dbl (bf16) staged in DRAM; broadcasts read from there.
        # exp/ln phases are batched so the ACT engine never swaps func tables
        # (Exp lives in set 0, Ln in set 5, Copy in every set).
        xdbd = drp.tile([3 * N, L], BF16, tag="xdbd")
        zf = [big.tile([128, L], BF16, name=f"zf{h}", tag=f"zf{h}")
              for h in range(2)]
        zb = [big.tile([128, L], BF16, name=f"zb{h}", tag=f"zb{h}")
              for h in range(2)]
        dT = zf    # softplus closes in place: dT aliases zf, dbT aliases zb
        ubT = [big.tile([128, L], BF16, name=f"ubT{h}", tag=f"ubT{h}")
               for h in range(2)]
        xsk = [big.tile([128, L], BF16, name=f"xsk{h}", tag=f"xsk{h}")
               for h in range(2)]

        for c in range(NLC):
            sl = slice(c * LC, (c + 1) * LC)
            # x_dbl^T chunk (64, LC) = Wxp @ x^T
            pd = mmp.tile([128, LC], FP32, tag="mmp", bufs=3)
            for h in range(2):
                nc.tensor.matmul(pd[0:PROJ, :], wxpT[h][:, :], xT[h][:, sl],
                                 start=(h == 0), stop=(h == 1))
            # fp32 delta_r rows for the dt matmul; bf16 B/C rows -> DRAM
            drc = wk.tile([R, LC], BF16, tag="drc", bufs=1)
            nc.vector.tensor_copy(drc[:, :], pd[0:R, :])
            bcc = wk.tile([PROJ, LC], BF16, tag="bcc")
            nc.vector.tensor_copy(bcc[:, :], pd[0:PROJ, :])
            nc.sync.dma_start(xdbd[:, sl], bcc[R:PROJ, :])
            # xb^T chunk (16, LC) = W_xbproj @ x^T  (FORWARD order)
            pb = mmp.tile([128, LC], FP32, tag="mmp", bufs=3)
            for h in range(2):
                nc.tensor.matmul(pb[0:R, :], wxbT[h][:, :], xT[h][:, sl],
                                 start=(h == 0), stop=(h == 1))
            xbc = wk.tile([R, LC], BF16, tag="xbc", bufs=1)
            nc.vector.tensor_copy(xbc[:, :], pb[0:R, :])
            for h in range(2):
                hsl = slice(h * 128, (h + 1) * 128)
                # z = W_dt @ delta_r^T (+b_dt later); staged to SBUF by Pool
                pz = mmp.tile([128, LC], FP32, tag="mmp", bufs=3)
                nc.tensor.matmul(pz[:, :], wdtT[:, hsl], drc[:, :],
                                 start=True, stop=True)
                nc.scalar.copy(zf[h][:, sl], pz[:, :])
                pz2 = mmp.tile([128, LC], FP32, tag="mmp", bufs=3)
                nc.tensor.matmul(pz2[:, :], wdtT[:, hsl], xbc[:, :],
                                 start=True, stop=True)
                nc.scalar.copy(zb[h][:, sl], pz2[:, :])

        # delta = ln(exp(z + b_dt) + 1) [softplus]: full-L exp then ln per
        # direction-half -- 8 ACT instrs, no act-table swaps mid-stream
        dbT = zb
        for h in range(2):
            nc.scalar.activation(zf[h][:, :], zf[h][:, :], AF.Exp,
                                 bias=bdt[h][:, 0:1])
            nc.scalar.activation(zb[h][:, :], zb[h][:, :], AF.Exp,
                                 bias=bdt[h][:, 0:1])
        for h in range(2):
            nc.scalar.activation(dT[h][:, :], zf[h][:, :], AF.Ln, bias=1.0)
            nc.scalar.activation(dbT[h][:, :], zb[h][:, :], AF.Ln, bias=1.0)
        for c in range(NLC):
            sl = slice(c * LC, (c + 1) * LC)
            rsl = slice(L - (c + 1) * LC, L - c * LC)
            for h in range(2):
                # ub = delta_b * x (forward order; read reversed later)
                nc.gpsimd.tensor_mul(ubT[h][:, sl], dbT[h][:, sl],
                                     xT[h][:, sl])
                # skip term (x + xf) * D_skip -> bf16 (matmul lhsT later)
                xs = wk.tile([128, LC], BF16, tag="ez")
                nc.gpsimd.tensor_add(xs[:, :], xT[h][:, sl],
                                     _rev_ap(xT[h][:, rsl]))
                nc.scalar.activation(xsk[h][:, sl], xs[:, :], AF.Copy,
                                     scale=dskip[h][:, 0:1])

        # ---- main scan loop ------------------------------------------
        def issue_reps(c, g):
            """Broadcast the (c, g) B/C n-rows to 128 partitions (prefetched
            one group ahead; rep tiles are double-buffered)."""
            sl_ = slice(c * LC, (c + 1) * LC)
            n0 = g * NG
            bf_rep = wk.tile([128, NG * LC], BF16, tag="bfr")
            bb_rep = wk.tile([128, NG * LC], BF16, tag="bbr")
            c_rep = wk.tile([128, NG * LC], BF16, tag="ccr")
            for rep, r0, qeng in ((bf_rep, n0, nc.sync),
                                  (bb_rep, N + n0, nc.sync),
                                  (c_rep, 2 * N + n0, nc.scalar)):
                s = xdbd[r0:r0 + NG, sl_]
                src_b = AP(s.tensor, s.offset,
                           [[0, 128], [s.ap[0][0], NG], [1, LC]])
                qeng.dma_start(_blk_ap(rep[:, :], NG, LC), src_b)
            return (bf_rep, bb_rep, c_rep)

        iters = [(c, g, h) for c in range(NLC) for g in range(G)
                 for h in range(2)]
        reps_of = {}
        carry = [[None, None], [None, None]]    # [g][h] -> carry cols tile
        u_cur = {}                              # (c, h) -> u chunk tile
        st = {}                                 # (c,g,h) -> stage-A tiles
        tree = {}                               # (c,g,h) -> y-part tile

        def ensure_reps(c, g):
            if (c, g) not in reps_of:
                reps_of[(c, g)] = issue_reps(c, g)
            return reps_of[(c, g)]

        def next_group(c, g):
            if g + 1 < G:
                return (c, g + 1)
            return (c + 1, 0) if c + 1 < NLC else None

        def stage_a(c, g, h):
            """a-cube exps (ACT), u mult, p/b products (DVE), badd (Pool)."""
            sl = slice(c * LC, (c + 1) * LC)
            rsl = slice(L - (c + 1) * LC, L - c * LC)
            n0 = g * NG
            bf_rep, bb_rep, c_rep = ensure_reps(c, g)
            if h == 0:
                ng = next_group(c, g)
                if ng:
                    ensure_reps(*ng)
            if (c, h) not in u_cur:
                ut = wk.tile([128, LC], BF16, tag=f"ut{h}", bufs=2)
                nc.vector.tensor_mul(ut[:, :], dT[h][:, sl], xT[h][:, sl])
                u_cur[(c, h)] = ut
            a_t = wk.tile([128, NG * LC], BF16, tag="at")
            for j in range(NG):
                n = n0 + j
                nc.scalar.activation(a_t[:, j * LC:(j + 1) * LC],
                                     dT[h][:, sl], AF.Exp,
                                     scale=aneg[h][:, n:n + 1])
            # ptm doubles as p-product scratch and later h*C tree buf
            ptm = wk.tile([128, NG * LC], BF16, tag="tm", bufs=3)
            b_t = wk.tile([128, NG * LC], BF16, tag="bt", bufs=3)
            nc.vector.tensor_tensor(_blk_ap(ptm[:, :], NG, LC),
                                    _rep_ap(u_cur[(c, h)][:, :], NG),
                                    _blk_ap(bf_rep[:, :], NG, LC), ALU.mult)
            nc.vector.tensor_tensor(_blk_ap(b_t[:, :], NG, LC),
                                    _rep_rev_ap(ubT[h][:, rsl], NG),
                                    _blk_ap(bb_rep[:, :], NG, LC), ALU.mult)
            nc.gpsimd.tensor_add(b_t[:, :], b_t[:, :], ptm[:, :])
            st[(c, g, h)] = (a_t, b_t, ptm, c_rep)

        def stage_b(c, g, h):
            """scans (DVE), carry snapshot + h*C tree reduce (Pool)."""
            a_t, b_t, ptm, c_rep = st.pop((c, g, h))
            h_t = wk.tile([128, NG * LC], BF16, tag="ht", bufs=2)
            for j in range(NG):
                js = slice(j * LC, (j + 1) * LC)
                if c == 0:
                    init = 0.0
                else:
                    init = carry[g][h][:, j:j + 1]
                nc.vector.tensor_tensor_scan(h_t[:, js], a_t[:, js],
                                             b_t[:, js], init,
                                             ALU.mult, ALU.add)
            if c < NLC - 1:
                cy = wk.tile([128, NG], BF16, tag=f"cy{g}{h}", bufs=2)
                nc.gpsimd.tensor_copy(
                    cy[:, :], AP(h_t.tensor, h_t[:, :].offset + LC - 1,
                                 [[h_t[:, :].ap[0][0], 128], [LC, NG]]))
                carry[g][h] = cy
            teng = nc.vector if (c == NLC - 1 and g == G - 1) else nc.gpsimd
            tmp = ptm
            teng.tensor_mul(tmp[:, :], h_t[:, :], c_rep[:, :])
            half = NG * LC // 2
            while half >= 2 * LC:
                teng.tensor_add(tmp[:, 0:half], tmp[:, 0:half],
                                tmp[:, half:2 * half])
                half //= 2
            yg = wk.tile([128, LC], BF16, tag=f"yg{g}{h}", bufs=2)
            teng.tensor_add(yg[:, :], tmp[:, 0:LC], tmp[:, LC:2 * LC])
            tree[(c, g, h)] = yg
            if (g, h) == (G - 1, 1):
                out_proj(c)

        def out_proj(c):
            # psum accumulates (yg0 + yg1 + xsk) @ W_out^T per l-subchunk
            for s in range(LC // LSUB):
                l0 = c * LC + s * LSUB
                ssl = slice(s * LSUB, (s + 1) * LSUB)
                pt = ops.tile([LSUB, D], FP32, tag="ops")
                terms = []
                for h in range(2):
                    terms += [(tree[(c, 0, h)][:, ssl], h),
                              (tree[(c, 1, h)][:, ssl], h),
                              (xsk[h][:, l0:l0 + LSUB], h)]
                for k, (term, h) in enumerate(terms):
                    nc.tensor.matmul(pt[:, :], term, woutT[h][:, :],
                                     start=(k == 0), stop=(k == len(terms) - 1))
                ot = wk.tile([LSUB, D], FP32, tag="osb")
                nc.scalar.copy(ot[:, :], pt[:, :])
                nc.sync.dma_start(out_d[l0:l0 + LSUB, :], ot[:, :])

        # two-iteration software-pipeline skew: A(i+2) is emitted before
        # B(i) so no engine blocks in-order behind a cross-engine handoff
        stage_a(*iters[0])
        stage_a(*iters[1])
        for k, it in enumerate(iters):
            if k + 2 < len(iters):
                stage_a(*iters[k + 2])
            stage_b(*it)


_NC_CACHE = {}  # v3


def _build():
    if "nc" in _NC_CACHE:
        return _NC_CACHE["nc"]
    nc = bacc.Bacc("TRN2", target_bir_lowering=False, debug=False,
                   num_devices=NCORES)
    x_d = nc.dram_tensor("x", [L, D], FP32, kind="ExternalInput").ap()
    wxpT_d = nc.dram_tensor("WxpT", [D, PROJ], BF16, kind="ExternalInput").ap()
    wxbT_d = nc.dram_tensor("WxbT", [D, R], BF16, kind="ExternalInput").ap()
    wdtT_d = nc.dram_tensor("WdtT", [R, D], BF16, kind="ExternalInput").ap()
    bdt_d = nc.dram_tensor("bdt", [D, 1], FP32, kind="ExternalInput").ap()
    aneg_d = nc.dram_tensor("Aneg", [D, N], FP32, kind="ExternalInput").ap()
    dskip_d = nc.dram_tensor("Dskip", [D, 1], FP32, kind="ExternalInput").ap()
    woutT_d = nc.dram_tensor("WoutT", [D, D], BF16, kind="ExternalInput").ap()
    eye_d = nc.dram_tensor("eye", [128, 128], FP32, kind="ExternalInput").ap()
    out_d = nc.dram_tensor("out", [L, D], FP32, kind="ExternalOutput").ap()
    io = (x_d, wxpT_d, wxbT_d, wdtT_d, bdt_d, aneg_d, dskip_d, woutT_d,
          eye_d, out_d)
    with tile.TileContext(nc) as tc:
        _emit(tc, nc, io)
    nc.compile()
    _NC_CACHE["nc"] = nc
    return nc


def host_prep(W_xproj, W_xbproj, W_dt, b_dt, A_log, D_skip, W_out):
    """Host-side input transforms shared by all cores."""
    import ml_dtypes

    return {
        "WxpT": np.ascontiguousarray(
            np.asarray(W_xproj, dtype=np.float32).T.astype(ml_dtypes.bfloat16)),
        "WxbT": np.ascontiguousarray(
            np.asarray(W_xbproj, dtype=np.float32).T.astype(ml_dtypes.bfloat16)),
        "WdtT": np.ascontiguousarray(
            np.asarray(W_dt, dtype=np.float32).T.astype(ml_dtypes.bfloat16)),
        "bdt": np.ascontiguousarray(
            np.asarray(b_dt, dtype=np.float32).reshape(D, 1)),
        "Aneg": np.ascontiguousarray(
            -np.exp(np.asarray(A_log, dtype=np.float32))),
        "Dskip": np.ascontiguousarray(
            np.asarray(D_skip, dtype=np.float32).reshape(D, 1)),
        "WoutT": np.ascontiguousarray(
            np.asarray(W_out, dtype=np.float32).T.astype(ml_dtypes.bfloat16)),
        "eye": np.eye(128, dtype=np.float32),
    }


def kernel(x, W_xproj, W_xbproj, W_dt, b_dt, A_log, D_skip, W_out, **profile_kw):
    nc = _build()
    shared = host_prep(W_xproj, W_xbproj, W_dt, b_dt, A_log, D_skip, W_out)
    xs = np.asarray(x, dtype=np.float32)
    in_maps = [{"x": np.ascontiguousarray(xs[b]), **shared} for b in range(NCORES)]
    res = bass_utils.run_bass_kernel_spmd(nc, in_maps, core_ids=list(range(NCORES)),
                                          **profile_kw)
    out = np.stack([res.results[b]["out"] for b in range(NCORES)], axis=0)
    kernel.last_result = res
    return out
